# revision 10
# baseline (speedup 1.0000x reference)
"""Contact-guided attention augment kernel for 8 trn2 NeuronCores.

Sharding: 8 cores = 4 head-pairs x 2 query-halves.
  core c: g = c % 4 -> heads (2g, 2g+1); sh = c // 4 -> query rows
  [sh*2048, (sh+1)*2048).
Each core's inputs are permuted so its own sequence half comes first,
making the device program identical across cores (pure SPMD).

Device layout: scores are computed transposed (keys on partitions,
queries on free dim), so softmax needs no attention transpose; the
softmax denominator comes from an extra ones-column in the V matmul
(M=65), and normalization folds into the PSUM->SBUF copy of the
attention output.
"""

import numpy as np

H = 8
D = 64
S = 4096
IN = 1024
NODE = 512
EPS = 1e-5

N_CORES = 8
N_G = 4          # head-pair groups
N_SH = 2         # sequence halves
SH_S = S // N_SH # 2048 queries per core
SBW = 1024       # query block width in main loop
N_SB = SH_S // SBW   # 2 query blocks
N_TC = S // 128      # 32 key chunks

_cache = {}


def _build():
    from contextlib import ExitStack

    from concourse import bacc, bass, mybir, tile
    from concourse.masks import make_identity

    f32 = mybir.dt.float32
    f32r = mybir.dt.float32r
    AF = mybir.ActivationFunctionType

    nc = bacc.Bacc(
        "TRN2",
        target_bir_lowering=False,
        debug=False,
        num_devices=N_CORES,
    )

    embT_d = nc.dram_tensor("embT", [IN, S], f32r, kind="ExternalInput")
    contactT_d = nc.dram_tensor("contactT", [S, SH_S], f32, kind="ExternalInput")
    wqqT_d = nc.dram_tensor("wqqT", [IN, 128], f32r, kind="ExternalInput")
    wkkT_d = nc.dram_tensor("wkkT", [IN, 128], f32r, kind="ExternalInput")
    wvvT_d = nc.dram_tensor("wvvT", [IN, 128], f32r, kind="ExternalInput")
    qscale_d = nc.dram_tensor("qscale", [128, 1], f32, kind="ExternalInput")
    qbias_d = nc.dram_tensor("qbias", [128, 1], f32, kind="ExternalInput")
    kbias_d = nc.dram_tensor("kbias", [128, 1], f32, kind="ExternalInput")
    vbias_d = nc.dram_tensor("vbias", [128, 1], f32, kind="ExternalInput")
    woTp_d = nc.dram_tensor("woTp", [64, 8, NODE], f32r, kind="ExternalInput")
    vecs_d = nc.dram_tensor("vecs", [3, NODE], f32, kind="ExternalInput")
    emb_res_d = nc.dram_tensor("emb_res", [NODE, NODE], f32, kind="ExternalInput")
    ones_d = nc.dram_tensor("ones", [128, 64], f32r, kind="ExternalInput")
    out_d = nc.dram_tensor("out", [NODE, NODE], f32, kind="ExternalOutput")

    def bcast(ap, n):
        return bass.AP(tensor=ap.tensor, offset=ap.offset, ap=[[0, n]] + list(ap.ap))

    with tile.TileContext(nc) as tc, ExitStack() as ctx:
        consts = ctx.enter_context(tc.tile_pool(name="consts", bufs=1))
        acts = ctx.enter_context(tc.tile_pool(name="acts", bufs=1))

        wqq = consts.tile([128, 8, 128], f32r)
        nc.sync.dma_start(out=wqq, in_=wqqT_d.ap().rearrange("(c p) m -> p c m", p=128))
        wkk = consts.tile([128, 8, 128], f32r)
        nc.sync.dma_start(out=wkk, in_=wkkT_d.ap().rearrange("(c p) m -> p c m", p=128))
        wvv = consts.tile([128, 8, 128], f32r)
        nc.sync.dma_start(out=wvv, in_=wvvT_d.ap().rearrange("(c p) m -> p c m", p=128))
        woTp = consts.tile([64, 8, NODE], f32r)
        nc.sync.dma_start(out=woTp, in_=woTp_d.ap())
        qscale = consts.tile([128, 1], f32)
        nc.sync.dma_start(out=qscale, in_=qscale_d.ap())
        qbias = consts.tile([128, 1], f32)
        nc.sync.dma_start(out=qbias, in_=qbias_d.ap())
        kbias = consts.tile([128, 1], f32)
        nc.sync.dma_start(out=kbias, in_=kbias_d.ap())
        vbias = consts.tile([128, 1], f32)
        nc.sync.dma_start(out=vbias, in_=vbias_d.ap())
        vecs = consts.tile([128, 3, NODE], f32)
        nc.sync.dma_start(out=vecs, in_=bcast(vecs_d.ap(), 128))
        emb_res = consts.tile([128, 4, NODE], f32)
        nc.sync.dma_start(
            out=emb_res, in_=emb_res_d.ap().rearrange("(c p) n -> p c n", p=128)
        )
        ident = consts.tile([128, 128], f32)
        make_identity(nc, ident)

        qT = acts.tile([128, SH_S], f32r)       # both heads stacked on partitions
        kT = acts.tile([128, S], f32r)
        v_nat = acts.tile([128, N_TC, 130], f32r)  # [t, chunk, (V_h0|1|V_h1|1)]
        x_all = acts.tile([128, 4, NODE], f32)    # pre-layernorm rows

        embT_r = embT_d.ap().rearrange("(c p) s -> p c s", p=128)

        # ---- phase A: projections ----
        with tc.tile_pool(name="pa_in", bufs=2) as pa_in, \
             tc.tile_pool(name="pa_ps", bufs=4, space="PSUM") as pa_ps, \
             tc.tile_pool(name="vT_pool", bufs=1) as vT_pool, \
             tc.tile_pool(name="pt_ps", bufs=2, space="PSUM") as pt_ps:
            vT = vT_pool.tile([128, S], f32)
            for sc in range(8):
                et = pa_in.tile([128, 8, 512], f32r)
                nc.sync.dma_start(out=et, in_=embT_r[:, :, sc * 512:(sc + 1) * 512])
                pk = pa_ps.tile([128, 512], f32, tag="pa", name="pk")
                pv = pa_ps.tile([128, 512], f32, tag="pa", name="pv")
                pq = None
                if sc < 4:
                    pq = pa_ps.tile([128, 512], f32, tag="pa", name="pq")
                for kc in range(8):
                    st, sp = kc == 0, kc == 7
                    nc.tensor.matmul(pk, wkk[:, kc, :],
                                     et[:, kc, :], start=st, stop=sp)
                    nc.tensor.matmul(pv, wvv[:, kc, :],
                                     et[:, kc, :], start=st, stop=sp)
                    if pq is not None:
                        nc.tensor.matmul(pq, wqq[:, kc, :],
                                         et[:, kc, :], start=st, stop=sp)
                sl = slice(sc * 512, (sc + 1) * 512)
                nc.scalar.activation(out=kT[:, sl], in_=pk, func=AF.Identity,
                                     bias=kbias)
                nc.scalar.activation(out=vT[:, sl], in_=pv, func=AF.Identity,
                                     bias=vbias)
                if pq is not None:
                    nc.scalar.activation(out=qT[:, sl], in_=pq, func=AF.Identity,
                                         bias=qbias, scale=qscale)

            # V into natural layout [t, d] (both heads), plus ones columns
            ones_r = ones_d.ap().rearrange("p (c one) -> p c one", one=1)
            nc.sync.dma_start(out=v_nat[:, :, 64:65], in_=ones_r[:, 0:N_TC, :])
            nc.sync.dma_start(out=v_nat[:, :, 129:130], in_=ones_r[:, 32:32 + N_TC, :])
            for blk in range(N_TC):
                pvt = pt_ps.tile([128, 128], f32)
                nc.tensor.transpose(pvt, vT[:, blk * 128:(blk + 1) * 128], ident)
                nc.scalar.activation(out=v_nat[:, blk, 0:64], in_=pvt[:, 0:64],
                                     func=AF.Copy)
                nc.scalar.activation(out=v_nat[:, blk, 65:129], in_=pvt[:, 64:128],
                                     func=AF.Copy)

        contactT_r = contactT_d.ap().rearrange("(c p) s -> p c s", p=128)

        # ---- phase B: attention ----
        with tc.tile_pool(name="ct", bufs=3) as ct_pool, \
             tc.tile_pool(name="exp0", bufs=2) as exp0, \
             tc.tile_pool(name="exp1", bufs=2) as exp1, \
             tc.tile_pool(name="ps", bufs=2, space="PSUM") as ps_pool, \
             tc.tile_pool(name="po", bufs=2, space="PSUM") as po_pool, \
             tc.tile_pool(name="fin", bufs=4) as fin:
            for sb in range(N_SB):
                po = [po_pool.tile([65, SBW], f32, tag="po", name=f"po{h}")
                      for h in range(2)]
                ex = [None, None]
                for tcx in range(N_TC):
                    ct = ct_pool.tile([128, SBW], f32)
                    nc.sync.dma_start(
                        out=ct,
                        in_=contactT_r[:, tcx, sb * SBW:(sb + 1) * SBW],
                    )
                    parity = tcx % 2
                    for hi in range(2):
                        hp = slice(hi * 64, (hi + 1) * 64)
                        if parity == 0:
                            ex[hi] = (exp0 if hi == 0 else exp1).tile(
                                [128, 2 * SBW], f32r, name=f"ex{hi}")
                        pss = ps_pool.tile([128, SBW], f32, tag="ps")
                        for half in range(2):
                            nc.tensor.matmul(
                                pss[:, half * 512:(half + 1) * 512],
                                kT[hp, tcx * 128:(tcx + 1) * 128],
                                qT[hp, sb * SBW + half * 512: sb * SBW + (half + 1) * 512],
                                start=True, stop=True, skip_group_check=True,
                            )
                        nc.vector.tensor_mul(
                            ex[hi][:, parity * SBW:(parity + 1) * SBW], pss, ct
                        )
                        if parity == 1:
                            nc.scalar.activation(out=ex[hi], in_=ex[hi], func=AF.Exp)
                            for t2 in (tcx - 1, tcx):
                                for half in range(2):
                                    nc.tensor.matmul(
                                        po[hi][:, half * 512:(half + 1) * 512],
                                        v_nat[:, t2, hi * 65:(hi + 1) * 65],
                                        ex[hi][:, (t2 % 2) * SBW + half * 512:
                                               (t2 % 2) * SBW + (half + 1) * 512],
                                        start=(t2 == 0), stop=(t2 == N_TC - 1),
                                        skip_group_check=True,
                                    )
                # normalize + output projection for this query block
                for hi in range(2):
                    rec = fin.tile([128, SBW], f32, tag="rec")
                    nc.vector.reciprocal(out=rec[64:65, :], in_=po[hi][64:65, :])
                    rec0 = fin.tile([1, SBW], f32, tag="rec0")
                    nc.sync.dma_start(out=rec0, in_=rec[64:65, :])
                    rb = fin.tile([64, SBW], f32, tag="rb")
                    nc.gpsimd.partition_broadcast(rb, rec0)
                    onorm = fin.tile([64, SBW], f32r, tag="onorm")
                    nc.vector.tensor_mul(onorm, po[hi][0:64, :], rb)
                    aug = ps_pool.tile([128, 512], f32, tag="ps")
                    on_r = onorm.rearrange("p (r j) -> p j r", j=8)
                    for j1 in range(8):
                        nc.tensor.matmul(
                            aug,
                            on_r[:, j1, :],
                            woTp[:, j1, :],
                            start=(j1 == 0), stop=(j1 == 7),
                            skip_group_check=True,
                        )
                    c2 = hi * 2 + sb
                    nc.vector.tensor_add(x_all[:, c2, :], aug, emb_res[:, c2, :])
                    nc.vector.tensor_add(x_all[:, c2, :], x_all[:, c2, :],
                                         vecs[:, 0, :])

        # ---- layernorm + store ----
        with tc.tile_pool(name="ln", bufs=4) as ln:
            out_r = out_d.ap().rearrange("(c p) n -> p c n", p=128)
            for c2 in range(4):
                stats = ln.tile([128, 6], f32, tag="stats")
                nc.vector.bn_stats(stats, x_all[:, c2, :])
                mv = ln.tile([128, 2], f32, tag="mv")
                nc.vector.bn_aggr(mv, stats)
                vv = ln.tile([128, 1], f32, tag="vv")
                nc.vector.tensor_scalar_add(vv, mv[:, 1:2], EPS)
                sq = ln.tile([128, 1], f32, tag="sq")
                nc.scalar.activation(out=sq, in_=vv, func=AF.Sqrt)
                rstd = ln.tile([128, 1], f32, tag="rstd")
                nc.vector.reciprocal(out=rstd, in_=sq)
                # one Newton step: r <- r * (1.5 - 0.5 * v * r^2)
                t0 = ln.tile([128, 1], f32, tag="t0")
                nc.vector.tensor_mul(t0, rstd, rstd)
                nc.vector.tensor_mul(t0, t0, vv)
                nc.vector.tensor_scalar_mul(t0, t0, -0.5)
                nc.vector.tensor_scalar_add(t0, t0, 1.5)
                nc.vector.tensor_mul(rstd, rstd, t0)
                nmu = ln.tile([128, 1], f32, tag="nmu")
                nc.vector.tensor_mul(nmu, mv[:, 0:1], rstd)
                nc.vector.tensor_scalar_mul(nmu, nmu, -1.0)
                xn = ln.tile([128, NODE], f32, tag="xn")
                nc.scalar.activation(out=xn, in_=x_all[:, c2, :], func=AF.Identity,
                                     bias=nmu, scale=rstd)
                nc.vector.tensor_mul(xn, xn, vecs[:, 1, :])
                nc.vector.tensor_add(xn, xn, vecs[:, 2, :])
                nc.sync.dma_start(out=out_r[:, c2, :], in_=xn)

    nc.compile()
    return nc


def make_in_maps(protT5_emb, contact_matrix, Wq, bq, Wk, bk, Wv, bv, Wc, Wo, bo,
                 gamma, beta):
    """Host-side sharding: slices, transposes, concats only (plus O(H)
    scalar constant folding for the Wc/sqrt(D) score scale)."""
    emb = np.asarray(protT5_emb, np.float32)
    contact = np.asarray(contact_matrix, np.float32)
    wc = np.asarray(Wc, np.float32).reshape(H)
    inv = 1.0 / np.sqrt(np.float32(D))

    embT = np.ascontiguousarray(emb.T)  # [IN, S]
    woTp = np.ascontiguousarray(
        np.asarray(Wo, np.float32).T.reshape(8, 64, NODE).transpose(1, 0, 2)
    )
    vecs = np.ascontiguousarray(np.stack([bo, gamma, beta]).astype(np.float32))

    # per-sh permuted embT and contactT
    embT_sh = []
    contactT_sh = []
    for sh in range(N_SH):
        own = slice(sh * SH_S, (sh + 1) * SH_S)
        oth = slice((1 - sh) * SH_S, (2 - sh) * SH_S)
        embT_sh.append(np.ascontiguousarray(
            np.concatenate([embT[:, own], embT[:, oth]], axis=1)))
        A = contact[own, :]  # [SH_S rows s, S cols t]
        contactT_sh.append(np.ascontiguousarray(
            np.concatenate([A[:, own].T, A[:, oth].T], axis=0)))

    in_maps = []
    for c in range(N_CORES):
        g, sh = c % N_G, c // N_G
        h0, h1 = 2 * g, 2 * g + 1
        s0, s1 = slice(h0 * D, (h0 + 1) * D), slice(h1 * D, (h1 + 1) * D)
        wqqT = np.ascontiguousarray(np.concatenate([Wq[s0], Wq[s1]]).T)
        wkkT = np.ascontiguousarray(np.concatenate([Wk[s0], Wk[s1]]).T)
        wvvT = np.ascontiguousarray(np.concatenate([Wv[s0], Wv[s1]]).T)
        qscale = np.concatenate([
            np.full(D, wc[h0] * inv, np.float32),
            np.full(D, wc[h1] * inv, np.float32)])[:, None]
        qbias = np.concatenate([
            np.asarray(bq, np.float32)[s0] * (wc[h0] * inv),
            np.asarray(bq, np.float32)[s1] * (wc[h1] * inv)])[:, None]
        kbias = np.concatenate([np.asarray(bk, np.float32)[s0],
                                np.asarray(bk, np.float32)[s1]])[:, None]
        vbias = np.concatenate([np.asarray(bv, np.float32)[s0],
                                np.asarray(bv, np.float32)[s1]])[:, None]
        r0 = slice(h0 * NODE + sh * 256, h0 * NODE + (sh + 1) * 256)
        r1 = slice(h1 * NODE + sh * 256, h1 * NODE + (sh + 1) * 256)
        emb_res = np.ascontiguousarray(
            np.concatenate([emb[r0, :NODE], emb[r1, :NODE]]))
        in_maps.append({
            "embT": embT_sh[sh],
            "contactT": contactT_sh[sh],
            "wqqT": wqqT,
            "wkkT": wkkT,
            "wvvT": wvvT,
            "qscale": np.ascontiguousarray(qscale),
            "qbias": np.ascontiguousarray(qbias),
            "kbias": np.ascontiguousarray(kbias),
            "vbias": np.ascontiguousarray(vbias),
            "woTp": woTp,
            "vecs": vecs,
            "emb_res": emb_res,
            "ones": np.ones((128, 64), np.float32),
        })
    return in_maps


def assemble(results):
    out = np.empty((S, NODE), np.float32)
    for c in range(N_CORES):
        g, sh = c % N_G, c // N_G
        h0, h1 = 2 * g, 2 * g + 1
        blk = results[c]["out"]
        out[h0 * NODE + sh * 256: h0 * NODE + (sh + 1) * 256] = blk[:256]
        out[h1 * NODE + sh * 256: h1 * NODE + (sh + 1) * 256] = blk[256:]
    return out


def kernel(**inputs):
    from concourse.bass_utils import run_bass_kernel_spmd

    if "nc" not in _cache:
        _cache["nc"] = _build()
    nc = _cache["nc"]
    in_maps = make_in_maps(**inputs)
    res = run_bass_kernel_spmd(nc, in_maps, list(range(N_CORES)))
    return assemble(res.results)
